# revision 21
# baseline (speedup 1.0000x reference)
"""Trainium2 Bass kernel for nn_Event_Critic_Net (dual-branch GAT critic).

Math: the reference only reads the GAT output at the LAST node of each
graph (graphs are 32 contiguous nodes), so only edges whose dst is a
graph's last node contribute.  For those edges the softmax-weighted
aggregation commutes with the linear projection W:

    out_g = sigmoid( (sum_n alpha[n] * x[n,:]) @ W + bias )
    alpha[n] = cnt[n]*exp(e[n]) / (sum_n cnt[n]*exp(e[n]) + 1e-16)
    e[n] = leaky_relu(x[n]. w_src + x[last(g)]. w_dst),  w_* = W @ att_*

Device pipeline per branch (graphs data-parallel over 8 cores):
  1. a_src: 64 matmuls, lhsT = xt chunk [128, 128] bf16 (feat-major,
     2-block packed: chunk c covers tiles c and 64+c), rhs = wv2.
  2. e = leaky_relu(asrc + adst_bcast); P = cnt * exp(e), computed in
     two HALVES (chunks 0-31 unlock tiles {0-31, 64-95} = xab pieces
     0 and 2; chunks 32-63 unlock pieces 1 and 3).
  3. z = P (*) xab per 32-tile piece: Pexp via 4x broadcast-copy then
     flat 2x bf16 multiply on DVE; two pieces go to GPSIMD directly.
  4. aggregation y = block-sums of z: stationary 32-col masks, 8
     accumulating MMs per col-group q (tile_position (0,32q)), each MM
     streams 4 contiguous tiles.  Output [128 graphs, 4*66] dense.
  5. normalize by denominator, 4 PE transposes -> ynT [64, 512];
     hT = W^T ynT; sigmoid(+bias).  Then sg_u*sg_d, mlp head, DMA out.
Host un-permutes the graph order at the end.
"""

import numpy as np
from contextlib import ExitStack

NC = 8            # cores
N = 131072        # nodes total
G = 4096          # graphs
NPG = 32          # nodes per graph
S = 64            # state size
H = 128           # hidden size
NPC = N // NC     # 16384 nodes per core
GPC = G // NC     # 512 graphs per core
T = NPC // 128    # 128 node-tiles per core
SA = 66           # x columns: 64 features | ones | zero pad
NPIECE = 4        # xab pieces per branch (32 tiles each)
TPP = T // NPIECE

_CACHE = {}


def _build_module():
    import concourse.tile as tile
    from concourse import bacc, mybir
    from concourse.alu_op_type import AluOpType as Alu

    f32 = mybir.dt.float32
    bf16 = mybir.dt.bfloat16
    Act = mybir.ActivationFunctionType
    AxX = mybir.AxisListType.X

    nc = bacc.Bacc("TRN2", target_bir_lowering=False, debug=False,
                   num_devices=NC)

    dram = {}

    def din(name, shape, dt=f32):
        dram[name] = nc.dram_tensor(name, shape, dt, kind="ExternalInput")

    for p in ("u", "d"):
        din(f"{p}_xab", [128, T * SA], bf16)
        din(f"{p}_xt", [128, NPC // 2], bf16)
    din("cb", [128, 1797], bf16)   # cstba | cstbb | xlast_u | xlast_d
    din("cf", [128, 392])          # cstf | cnt_u | cnt_d
    out_dram = nc.dram_tensor("out", [1, GPC], f32, kind="ExternalOutput")

    with tile.TileContext(nc) as tc, ExitStack() as ctx:
        const = ctx.enter_context(tc.tile_pool(name="const", bufs=1))
        xp = ctx.enter_context(tc.tile_pool(name="xp", bufs=2))
        wk = ctx.enter_context(tc.tile_pool(name="wk", bufs=2))
        ps1 = ctx.enter_context(tc.tile_pool(name="ps1", bufs=1, space="PSUM"))
        ps2 = ctx.enter_context(tc.tile_pool(name="ps2", bufs=2, space="PSUM"))

        # ---- constants / small inputs (2 consolidated tensors) ----
        cb = const.tile([128, 1797], bf16, tag="cb")
        nc.scalar.dma_start(cb[:], dram["cb"].ap())
        cf = const.tile([128, 392], f32, tag="cf")
        nc.sync.dma_start(cf[:], dram["cf"].ap())

        eps = cf[:, 4:5]
        mlpb = cf[0:1, 5:6]
        biases = {"u": cf[:, 6:7], "d": cf[:, 7:8]}
        identf = cf[:, 8:136]
        cnts = {"u": cf[:, 136:264], "d": cf[:, 264:392]}
        Qm = cb[0:4, 0:128]
        wdsts = {"u": cb[:, 128:384], "d": cb[:, 384:640]}
        wv2s = {"u": cb[:, 640:642], "d": cb[:, 642:644]}
        masks = cb[:, 644:900]
        Ws = {"u": cb[0:64, 900:1028], "d": cb[0:64, 1028:1156]}
        mlpW = cb[:, 1156:1157]
        identb = cb[:, 1157:1285]
        xlasts = {"u": cb[:, 1285:1541], "d": cb[:, 1541:1797]}

        st = {"u": {}, "d": {}}
        for p in ("u", "d"):
            s = st[p]
            s["cnt"] = cnts[p]
            s["xl"] = xlasts[p]
            s["xt"] = [xp.tile([128, 4096], bf16, tag=f"xt{h}",
                               name=f"xt{h}_{p}") for h in range(2)]
            s["x"] = [xp.tile([128, TPP * SA], bf16, tag=f"x{j}",
                              name=f"x{j}_{p}") for j in range(NPIECE)]

        def ldxt(eng, p, h):
            t = st[p]["xt"][h]
            eng.dma_start(t[:], dram[f"{p}_xt"].ap()[:, 4096 * h:
                                                     4096 * (h + 1)])

        def ldx(eng, p, j):
            t = st[p]["x"][j]
            eng.dma_start(t[:], dram[f"{p}_xab"].ap()[:, TPP * SA * j:
                                                      TPP * SA * (j + 1)])

        # <=8 HW dma_starts (HWDGE sem ring is 8 — avoid reuse stalls).
        # sync queue gets starved under load -> only consts + last pieces.
        ldxt(nc.scalar, "u", 0)
        ldxt(nc.scalar, "d", 0)
        ldx(nc.scalar, "u", 2)
        ldx(nc.scalar, "d", 2)
        ldx(nc.sync, "u", 3)
        ldx(nc.sync, "d", 3)
        ldx(nc.gpsimd, "u", 0)
        ldxt(nc.gpsimd, "u", 1)
        ldx(nc.gpsimd, "d", 0)
        ldxt(nc.gpsimd, "d", 1)
        ldx(nc.gpsimd, "u", 1)
        ldx(nc.gpsimd, "d", 1)

        # ---- a_src matmuls (grouped by xt half) + a_dst broadcast ----
        for p in ("u", "d"):
            s = st[p]
            asps = ps2.tile([128, T], f32, tag="asps", name=f"asps_{p}")
            s["asps"] = asps

            # a_dst at last nodes: mult+reduce, transpose, broadcast
            tmp4 = wk.tile([128, 4 * S], bf16, tag="tmp4")
            nc.vector.tensor_tensor(tmp4[:], s["xl"][:], wdsts[p],
                                    op=Alu.mult)
            adst = wk.tile([128, 4], f32, tag="adst")
            nc.vector.tensor_reduce(
                adst[:], tmp4[:].rearrange("p (j s) -> p j s", s=S),
                axis=AxX, op=Alu.add)
            tp = ps1.tile([4, 128], f32, tag="mix")
            nc.tensor.transpose(tp[:], adst[:], identf)
            adT = wk.tile([4, 128], bf16, tag="adT")
            nc.vector.tensor_copy(adT[:], tp[:])
            adbc_ps = ps1.tile([128, T], f32, tag="adbc")
            nc.tensor.matmul(adbc_ps[:], Qm, adT[:], start=True, stop=True)
            adbc = wk.tile([128, T], f32, tag="adbcs", name=f"adbcs_{p}")
            s["adbc"] = adbc
            nc.scalar.activation(adbc[:], adbc_ps[:], Act.Copy)
            s["P"] = wk.tile([128, T], bf16, tag="P", name=f"P_{p}")
            s["asrc"] = wk.tile([128, T], f32, tag="asrc", name=f"asrc_{p}")

        for h in range(2):
            for p in ("u", "d"):
                s = st[p]
                xth = s["xt"][h]
                for cc in range(32):
                    c = 32 * h + cc
                    nc.tensor.matmul(
                        s["asps"][0:128, 2 * c:2 * c + 2],
                        xth[:, 128 * cc:128 * cc + 128],
                        wv2s[p],
                        start=True, stop=True)

        # ---- e, P per (branch, half) ----
        for p in ("u", "d"):
            s = st[p]
            asps, asrc, cnt, adbc = s["asps"], s["asrc"], s["cnt"], s["adbc"]
            aspv = asps[:].rearrange("p (c j) -> p j c", j=2)
            for h in range(2):
                cs = slice(64 * h, 64 * h + 64)
                hs = slice(32 * h, 32 * h + 32)
                hs2 = slice(64 + 32 * h, 96 + 32 * h)
                nc.scalar.activation(asrc[:, hs], aspv[:, 0, 32 * h:
                                                       32 * h + 32], Act.Copy)
                nc.scalar.activation(asrc[:, hs2], aspv[:, 1, 32 * h:
                                                        32 * h + 32], Act.Copy)
                for sl in (hs, hs2):
                    z0 = wk.tile([128, 32], f32, tag="z0")
                    nc.vector.tensor_tensor(z0[:], asrc[:, sl], adbc[:, sl],
                                            op=Alu.add)
                    e = wk.tile([128, 32], f32, tag="e")
                    nc.vector.scalar_tensor_tensor(
                        e[:], z0[:], 0.2, z0[:], op0=Alu.mult, op1=Alu.max)
                    ex = wk.tile([128, 32], f32, tag="ex")
                    nc.scalar.activation(ex[:], e[:], Act.Exp)
                    nc.vector.tensor_tensor(s["P"][:, sl], ex[:], cnt[:, sl],
                                            op=Alu.mult)

        # ---- z = P (*) xab (half-pieces; late pieces split DVE+gps) ----
        HTP = TPP // 2      # 16 tiles per half-piece
        gps_half = {("u", 1, 1), ("u", 2, 1), ("d", 1, 1), ("d", 2, 1)}
        for p in ("u", "d"):
            s = st[p]
            s["z"] = []
            for j in range(NPIECE):
                zh = []
                for k in range(2):
                    z = wk.tile([128, HTP * SA], bf16, tag=f"z{j}{k}",
                                name=f"z{j}{k}_{p}")
                    zh.append(z)
                    ts = slice(TPP * j + HTP * k, TPP * j + HTP * (k + 1))
                    eng = (nc.gpsimd if (p, j, k) in gps_half
                           else nc.vector)
                    eng.tensor_tensor(
                        z[:].rearrange("q (t s) -> q t s", s=SA),
                        s["x"][j][:].rearrange(
                            "q (t s) -> q t s", s=SA)[:, HTP * k:
                                                      HTP * (k + 1), :],
                        s["P"][:, ts].broadcast_to((128, HTP, SA)),
                        op=Alu.mult)
                s["z"].append(zh)

        for p in ("u", "d"):
            s = st[p]
            yps = ps2.tile([128, 4 * SA], f32, tag="yps", name=f"yps_{p}")
            for q in range(4):
                for m in range(8):
                    zq = s["z"][q][m // 4]
                    mm = m % 4
                    nc.tensor.matmul(
                        yps[32 * q:32 * q + 32, :],
                        masks[:, 32 * m:32 * m + 32],
                        zq[:, SA * 4 * mm:SA * 4 * (mm + 1)],
                        start=(m == 0), stop=(m == 7),
                        tile_position=(0, 32 * q))

            # normalize: denom at cols 66t'+64
            ysb = wk.tile([128, 4 * SA], f32, tag="ysb")
            nc.scalar.activation(ysb[:], yps[:], Act.Copy)
            dn = wk.tile([128, 4], f32, tag="dn")
            nc.vector.tensor_scalar(
                dn[:], ysb[:].rearrange("p (t s) -> p t s", s=SA)[:, :, S],
                eps, None, op0=Alu.add)
            rp = wk.tile([128, 4], f32, tag="rp")
            nc.vector.reciprocal_approx_fast(rp[:], dn[:])
            ynrm = wk.tile([128, 4 * S], bf16, tag="ynrm")
            nc.vector.tensor_tensor(
                ynrm[:].rearrange("p (t s) -> p t s", s=S),
                ysb[:].rearrange("p (t s) -> p t s", s=SA)[:, :, 0:S],
                rp[:].broadcast_to((128, 4, S)),
                op=Alu.mult)

            # transpose 4x [128, 64] -> ynT [64, 512]
            ytp = ps1.tile([64, GPC], bf16, tag="ytp")
            for t4 in range(4):
                nc.tensor.transpose(
                    ytp[:, 128 * t4:128 * (t4 + 1)],
                    ynrm[:, S * t4:S * (t4 + 1)],
                    identb, tile_position=(0, 0))
            ynT = wk.tile([64, GPC], bf16, tag="ynT")
            nc.vector.tensor_copy(ynT[:], ytp[:])

            # project + bias + sigmoid
            hT = ps1.tile([H, GPC], f32, tag="hT")
            nc.tensor.matmul(hT[:], Ws[p], ynT[:], start=True, stop=True)
            sg = wk.tile([H, GPC], bf16, tag="sig", name=f"sig_{p}")
            nc.scalar.activation(sg[:], hT[:], Act.Sigmoid, bias=biases[p])
            s["sig"] = sg

        # ---- combine branches + MLP head ----
        prod = wk.tile([H, GPC], bf16, tag="prod")
        nc.vector.tensor_tensor(prod[:], st["u"]["sig"][:], st["d"]["sig"][:],
                                op=Alu.mult)
        o_ps = ps1.tile([1, GPC], f32, tag="mix")
        nc.tensor.matmul(o_ps[:], mlpW, prod[:], start=True, stop=True)
        o_sb = wk.tile([1, GPC], f32, tag="o_sb")
        nc.vector.tensor_scalar(
            o_sb[:], o_ps[:], mlpb, None, op0=Alu.add)
        nc.sync.dma_start(out_dram.ap(), o_sb[:])

    nc.compile()
    return nc


def _get_module():
    if "nc" not in _CACHE:
        _CACHE["nc"] = _build_module()
    return _CACHE["nc"]


def _perm():
    # device col c = 128*t' + 32*q + 4*m + b  ->  tile 32q+4m+t', block b
    c = np.arange(GPC)
    tp, r = c // 128, c % 128
    q, mm, b = r // 32, (r % 32) // 4, r % 4
    return 4 * (32 * q + 4 * mm + tp) + b


def _prep_branch(x, ei, W, att_src, att_dst, bias):
    """Host-side sharding + graph-format prep for one branch."""
    import ml_dtypes
    bf = ml_dtypes.bfloat16
    x = np.asarray(x, np.float32)
    src = np.asarray(ei[0]).astype(np.int64)
    dst = np.asarray(ei[1]).astype(np.int64)
    W = np.asarray(W, np.float32)
    w_src = (W @ np.asarray(att_src, np.float32)).astype(np.float32)
    w_dst = (W @ np.asarray(att_dst, np.float32)).astype(np.float32)

    valid = (dst % NPG) == (NPG - 1)
    cnt = np.bincount(src[valid], minlength=N).astype(np.float32)

    per_core = []
    for c in range(NC):
        xs = x[c * NPC:(c + 1) * NPC]
        xab = np.zeros((T, 128, SA), np.float32)
        xab[:, :, :S] = xs.reshape(T, 128, S)
        xab[:, :, S] = 1.0
        xab = np.ascontiguousarray(
            xab.transpose(1, 0, 2).reshape(128, T * SA)).astype(bf)
        # xt[64k+s, m] = x[8192k + m, s]
        xtv = xs.reshape(2, NPC // 2, S).transpose(0, 2, 1)
        xtv = np.ascontiguousarray(xtv.reshape(128, NPC // 2)).astype(bf)
        cnt_t = np.ascontiguousarray(
            cnt[c * NPC:(c + 1) * NPC].reshape(T, 128).T)
        xlast = np.ascontiguousarray(
            xs[NPG - 1::NPG].reshape(128, 4 * S)).astype(bf)
        per_core.append({"xab": xab, "xt": xtv, "cnt": cnt_t, "xlast": xlast})

    wv2 = np.zeros((128, 2), np.float32)
    wv2[:S, 0] = w_src
    wv2[S:, 1] = w_src
    wdst_rep = np.broadcast_to(w_dst, (128, 4, S)).reshape(128, 4 * S)
    shared = {
        "wv2": wv2,
        "wdst": wdst_rep.astype(np.float32),
        "W": W,
        "bias": np.asarray(bias, np.float32).reshape(H, 1),
    }
    return per_core, shared


def _build_in_maps(inputs):
    import ml_dtypes
    bf = ml_dtypes.bfloat16
    pcs = {}
    shareds = {}
    pcs["u"], shareds["u"] = _prep_branch(
        inputs["up_x"], inputs["up_edge_index"], inputs["up_W"],
        inputs["up_att_src"], inputs["up_att_dst"], inputs["up_bias"])
    pcs["d"], shareds["d"] = _prep_branch(
        inputs["down_x"], inputs["down_edge_index"], inputs["down_W"],
        inputs["down_att_src"], inputs["down_att_dst"], inputs["down_bias"])

    pp = np.arange(128)
    cf = np.zeros((128, 392), np.float32)
    cf[:, 4] = 1e-16                               # eps
    cf[0, 5] = float(np.asarray(inputs["mlp_b"]).reshape(-1)[0])
    cf[:, 6] = shareds["u"]["bias"][:, 0]
    cf[:, 7] = shareds["d"]["bias"][:, 0]
    cf[:, 8:136] = np.eye(128, dtype=np.float32)

    cbase = np.zeros((128, 1797), np.float32)
    cbase[pp // 32, pp] = 1.0                      # Qm
    cbase[:, 128:384] = shareds["u"]["wdst"]
    cbase[:, 384:640] = shareds["d"]["wdst"]
    cbase[:, 640:642] = shareds["u"]["wv2"]
    cbase[:, 642:644] = shareds["d"]["wv2"]
    for m in range(8):
        cbase[pp, 644 + 32 * m + 4 * m + pp // 32] = 1.0
    cbase[0:64, 900:1028] = shareds["u"]["W"]
    cbase[0:64, 1028:1156] = shareds["d"]["W"]
    cbase[:, 1156] = np.asarray(inputs["mlp_W"], np.float32).reshape(H)
    cbase[:, 1157:1285] = np.eye(128, dtype=np.float32)

    in_maps = []
    for c in range(NC):
        cfc = cf.copy()
        cfc[:, 136:264] = pcs["u"][c]["cnt"]
        cfc[:, 264:392] = pcs["d"][c]["cnt"]
        cbc = cbase.copy()
        cbc[:, 1285:1541] = pcs["u"][c]["xlast"].astype(np.float32)
        cbc[:, 1541:1797] = pcs["d"][c]["xlast"].astype(np.float32)
        m = {"cf": cfc, "cb": cbc.astype(bf)}
        for p in ("u", "d"):
            m[f"{p}_xab"] = pcs[p][c]["xab"]
            m[f"{p}_xt"] = pcs[p][c]["xt"]
        in_maps.append(m)
    return in_maps


def kernel(**inputs):
    from concourse.bass_utils import run_bass_kernel_spmd

    nc = _get_module()
    in_maps = _build_in_maps(inputs)
    res = run_bass_kernel_spmd(nc, in_maps, core_ids=list(range(NC)))
    perm = _perm()
    out = np.empty((NC, GPC), np.float32)
    for c, r in enumerate(res.results):
        out[c, perm] = np.asarray(r["out"], np.float32).reshape(GPC)
    return out.reshape(G, 1)


# revision 23
# speedup vs baseline: 1.0990x; 1.0990x over previous
"""Trainium2 Bass kernel for nn_Event_Critic_Net (dual-branch GAT critic).

Math: the reference only reads the GAT output at the LAST node of each
graph (graphs are 32 contiguous nodes), so only edges whose dst is a
graph's last node contribute.  For those edges the softmax-weighted
aggregation commutes with the linear projection W:

    out_g = sigmoid( (sum_n alpha[n] * x[n,:]) @ W + bias )
    alpha[n] = cnt[n]*exp(e[n]) / (sum_n cnt[n]*exp(e[n]) + 1e-16)
    e[n] = leaky_relu(x[n]. w_src + x[last(g)]. w_dst),  w_* = W @ att_*

Device pipeline per branch (graphs data-parallel over 8 cores):
  1. a_src: 64 matmuls, lhsT = xt chunk [128, 128] bf16 (feat-major,
     2-block packed: chunk c covers tiles c and 64+c), rhs = wv2.
     Emitted per xt HALF with per-half PSUM tiles so dependencies are
     half-granular.
  2. e = leaky_relu(asrc + adst_bcast); P = cnt * exp(e), per half
     (half h covers tiles {32h..32h+32, 64+32h..96+32h} = xab pieces
     h and h+2).
  3. z = P (*) xab per 16-tile half-piece (stride-0 broadcast multiply,
     bf16); the latest-arriving halves run on GPSIMD, rest on DVE.
  4. aggregation y = block-sums of z: stationary 32-col masks, 8
     accumulating MMs per col-group q (tile_position (0,32q)), each MM
     streams 4 contiguous tiles.  Output [128 graphs, 4*66] dense.
  5. normalize by denominator (ones-column), 4 PE transposes ->
     ynT [64, 512]; hT = W^T ynT; sigmoid(+bias).
Then sg_u * sg_d, mlp head, DMA out.  Host un-permutes graph order.

All engine queues are strict FIFO, so every instruction is emitted in
expected-data-arrival order to avoid head-of-line blocking.
"""

import numpy as np
from contextlib import ExitStack

NC = 8            # cores
N = 131072        # nodes total
G = 4096          # graphs
NPG = 32          # nodes per graph
S = 64            # state size
H = 128           # hidden size
NPC = N // NC     # 16384 nodes per core
GPC = G // NC     # 512 graphs per core
T = NPC // 128    # 128 node-tiles per core
SA = 66           # x columns: 64 features | ones | zero pad
NPIECE = 4        # xab pieces per branch (32 tiles each)
TPP = T // NPIECE
HTP = TPP // 2    # 16 tiles per half-piece

_CACHE = {}


def _build_module():
    import concourse.tile as tile
    from concourse import bacc, mybir
    from concourse.alu_op_type import AluOpType as Alu

    f32 = mybir.dt.float32
    bf16 = mybir.dt.bfloat16
    Act = mybir.ActivationFunctionType
    AxX = mybir.AxisListType.X

    nc = bacc.Bacc("TRN2", target_bir_lowering=False, debug=False,
                   num_devices=NC)

    dram = {}

    def din(name, shape, dt=f32):
        dram[name] = nc.dram_tensor(name, shape, dt, kind="ExternalInput")

    for p in ("u", "d"):
        din(f"{p}_xab", [128, T * SA], bf16)
        din(f"{p}_xt", [128, NPC // 2], bf16)
    din("cb", [128, 1797], bf16)   # consts | xlast_u | xlast_d
    din("cf", [128, 392])          # cstf | cnt_u | cnt_d
    out_dram = nc.dram_tensor("out", [1, GPC], f32, kind="ExternalOutput")

    with tile.TileContext(nc) as tc, ExitStack() as ctx:
        const = ctx.enter_context(tc.tile_pool(name="const", bufs=1))
        xp = ctx.enter_context(tc.tile_pool(name="xp", bufs=2))
        wk = ctx.enter_context(tc.tile_pool(name="wk", bufs=2))
        ps1 = ctx.enter_context(tc.tile_pool(name="ps1", bufs=1, space="PSUM"))
        ps2 = ctx.enter_context(tc.tile_pool(name="ps2", bufs=2, space="PSUM"))

        cb = const.tile([128, 1797], bf16, tag="cb")
        nc.scalar.dma_start(cb[:], dram["cb"].ap())
        cf = const.tile([128, 392], f32, tag="cf")
        nc.sync.dma_start(cf[:], dram["cf"].ap())

        eps = cf[:, 4:5]
        mlpb = cf[0:1, 5:6]
        biases = {"u": cf[:, 6:7], "d": cf[:, 7:8]}
        identf = cf[:, 8:136]
        cnts = {"u": cf[:, 136:264], "d": cf[:, 264:392]}
        Qm = cb[0:4, 0:128]
        wdsts = {"u": cb[:, 128:384], "d": cb[:, 384:640]}
        wv2s = {"u": cb[:, 640:642], "d": cb[:, 642:644]}
        masks = cb[:, 644:900]
        Ws = {"u": cb[0:64, 900:1028], "d": cb[0:64, 1028:1156]}
        mlpW = cb[:, 1156:1157]
        identb = cb[:, 1157:1285]
        xlasts = {"u": cb[:, 1285:1541], "d": cb[:, 1541:1797]}

        st = {"u": {}, "d": {}}
        for p in ("u", "d"):
            s = st[p]
            s["xt"] = [xp.tile([128, 4096], bf16, tag=f"xt{h}",
                               name=f"xt{h}_{p}") for h in range(2)]
            s["x"] = [xp.tile([128, TPP * SA], bf16, tag=f"x{j}",
                              name=f"x{j}_{p}") for j in range(NPIECE)]

        def ldxt(eng, p, h):
            t = st[p]["xt"][h]
            eng.dma_start(t[:], dram[f"{p}_xt"].ap()[:, 4096 * h:
                                                     4096 * (h + 1)])

        def ldx(eng, p, j):
            t = st[p]["x"][j]
            eng.dma_start(t[:], dram[f"{p}_xab"].ap()[:, TPP * SA * j:
                                                      TPP * SA * (j + 1)])

        # queue map (sync starves under load -> only consts + p3 pieces)
        ldxt(nc.scalar, "u", 0)
        ldxt(nc.scalar, "d", 0)
        ldx(nc.scalar, "u", 1)
        ldx(nc.scalar, "d", 1)
        ldx(nc.scalar, "d", 2)
        ldxt(nc.gpsimd, "u", 1)
        ldxt(nc.gpsimd, "d", 1)
        ldx(nc.gpsimd, "u", 0)
        ldx(nc.gpsimd, "d", 0)
        ldx(nc.gpsimd, "u", 2)
        ldx(nc.sync, "u", 3)
        ldx(nc.sync, "d", 3)

        # ---- a_dst broadcast (small, early: needs cb/cf only) ----
        for p in ("u", "d"):
            s = st[p]
            tmp4 = wk.tile([128, 4 * S], bf16, tag="tmp4")
            nc.vector.tensor_tensor(tmp4[:], xlasts[p], wdsts[p],
                                    op=Alu.mult)
            adst = wk.tile([128, 4], f32, tag="adst")
            nc.vector.tensor_reduce(
                adst[:], tmp4[:].rearrange("p (j s) -> p j s", s=S),
                axis=AxX, op=Alu.add)
            tp = ps1.tile([4, 128], f32, tag="mix")
            nc.tensor.transpose(tp[:], adst[:], identf)
            adT = wk.tile([4, 128], bf16, tag="adT")
            nc.vector.tensor_copy(adT[:], tp[:])
            adbc_ps = ps1.tile([128, T], f32, tag="adbc")
            nc.tensor.matmul(adbc_ps[:], Qm, adT[:], start=True, stop=True)
            adbc = wk.tile([128, T], f32, tag="adbcs", name=f"adbcs_{p}")
            s["adbc"] = adbc
            nc.scalar.activation(adbc[:], adbc_ps[:], Act.Copy)
            s["P"] = [wk.tile([128, 64], bf16, tag=f"P{h}",
                              name=f"P{h}_{p}") for h in range(2)]
            s["asps"] = [None, None]

        def asps_half(p, h):
            """32 a_src matmuls for xt half h (chunks 32h..32h+32)."""
            s = st[p]
            aps = ps1.tile([128, 64], f32, tag=f"as{h}", name=f"as{h}_{p}")
            s["asps"][h] = aps
            xth = s["xt"][h]
            for cc in range(32):
                nc.tensor.matmul(
                    aps[0:128, 2 * cc:2 * cc + 2],
                    xth[:, 128 * cc:128 * cc + 128],
                    wv2s[p],
                    start=True, stop=True)

        def e_half(p, h):
            """e/P for half h: tiles {32h..32h+32} (j=0) and
            {64+32h..96+32h} (j=1); P[h] cols = [j*32 + t]."""
            s = st[p]
            aps = s["asps"][h]
            aspv = aps[:].rearrange("p (c j) -> p j c", j=2)
            asr = wk.tile([128, 64], f32, tag="asr")
            nc.scalar.activation(asr[:, 0:32], aspv[:, 0, :], Act.Copy)
            nc.scalar.activation(asr[:, 32:64], aspv[:, 1, :], Act.Copy)
            # adbc/cnt cols {32h..32h+32} U {64+32h..96+32h}: strided view
            adv = st[p]["adbc"][:].rearrange(
                "p (k t) -> p k t", k=4)[:, h::2, :]
            cnv = cnts[p].rearrange("p (k t) -> p k t", k=4)[:, h::2, :]
            z0 = wk.tile([128, 64], f32, tag="z0")
            nc.vector.tensor_tensor(
                z0[:].rearrange("p (j t) -> p j t", j=2), asr[:].rearrange(
                    "p (j t) -> p j t", j=2), adv, op=Alu.add)
            e = wk.tile([128, 64], f32, tag="e")
            nc.vector.scalar_tensor_tensor(
                e[:], z0[:], 0.2, z0[:], op0=Alu.mult, op1=Alu.max)
            ex = wk.tile([128, 64], f32, tag="ex")
            nc.scalar.activation(ex[:], e[:], Act.Exp)
            nc.vector.tensor_tensor(
                s["P"][h][:].rearrange("p (j t) -> p j t", j=2),
                ex[:].rearrange("p (j t) -> p j t", j=2), cnv, op=Alu.mult)

        def z_half(p, j, k, eng):
            """z for piece j half k (16 tiles) on engine eng."""
            s = st[p]
            z = wk.tile([128, HTP * SA], bf16, tag=f"z{j}{k}",
                        name=f"z{j}{k}_{p}")
            s["z"][j][k] = z
            # piece j uses P half (j % 2), cols (j // 2)*32 + [0, 32)
            ph = s["P"][j % 2][:, 32 * (j // 2) + 16 * k:
                               32 * (j // 2) + 16 * (k + 1)]
            eng.tensor_tensor(
                z[:].rearrange("q (t s) -> q t s", s=SA),
                s["x"][j][:].rearrange(
                    "q (t s) -> q t s", s=SA)[:, HTP * k:HTP * (k + 1), :],
                ph.broadcast_to((128, HTP, SA)),
                op=Alu.mult)

        def agg_q(p, q):
            """8 accumulating mask-MMs for col-group q of branch p."""
            s = st[p]
            for m in range(8):
                zq = s["z"][q][m // 4]
                mm = m % 4
                nc.tensor.matmul(
                    s["yps"][32 * q:32 * q + 32, :],
                    masks[:, 32 * m:32 * m + 32],
                    zq[:, SA * 4 * mm:SA * 4 * (mm + 1)],
                    start=(m == 0), stop=(m == 7),
                    tile_position=(0, 32 * q))

        for p in ("u", "d"):
            st[p]["z"] = [[None, None] for _ in range(NPIECE)]
            st[p]["yps"] = ps2.tile([128, 4 * SA], f32, tag="yps",
                                    name=f"yps_{p}")

        # ---- pipelined emission, ordered by expected data arrival ----
        # est: xt_u h1 ~7, xt_u h0 ~10, xt_d h1 ~14, xt_d h0 ~17,
        #      xab: u0 ~18, d3/u1 ~20, d0/u3 ~21, u2/d1 ~24, d2 ~27
        V, GP = nc.vector, nc.gpsimd
        asps_half("u", 1)
        asps_half("u", 0)
        e_half("u", 1)
        e_half("u", 0)
        asps_half("d", 1)
        asps_half("d", 0)
        e_half("d", 1)
        e_half("d", 0)
        z_half("u", 0, 0, V); z_half("u", 0, 1, V)
        agg_q("u", 0)
        z_half("u", 1, 0, V); z_half("d", 3, 0, V)
        z_half("u", 1, 1, GP)
        agg_q("u", 1)
        z_half("d", 0, 0, V); z_half("d", 0, 1, V)
        agg_q("d", 0)
        z_half("u", 3, 0, V); z_half("u", 3, 1, GP)
        agg_q("u", 3)
        z_half("d", 3, 1, V)
        agg_q("d", 3)
        z_half("u", 2, 0, V); z_half("u", 2, 1, GP)
        agg_q("u", 2)
        z_half("d", 1, 0, V); z_half("d", 1, 1, GP)
        agg_q("d", 1)
        z_half("d", 2, 0, V); z_half("d", 2, 1, V)
        agg_q("d", 2)

        # ---- normalize, transpose, project, sigmoid (u then d) ----
        for p in ("u", "d"):
            s = st[p]
            ysb = wk.tile([128, 4 * SA], f32, tag="ysb")
            nc.scalar.activation(ysb[:], s["yps"][:], Act.Copy)
            dn = wk.tile([128, 4], f32, tag="dn")
            nc.vector.tensor_scalar(
                dn[:], ysb[:].rearrange("p (t s) -> p t s", s=SA)[:, :, S],
                eps, None, op0=Alu.add)
            rp = wk.tile([128, 4], f32, tag="rp")
            nc.vector.reciprocal_approx_fast(rp[:], dn[:])
            ynrm = wk.tile([128, 4 * S], bf16, tag="ynrm")
            nc.vector.tensor_tensor(
                ynrm[:].rearrange("p (t s) -> p t s", s=S),
                ysb[:].rearrange("p (t s) -> p t s", s=SA)[:, :, 0:S],
                rp[:].broadcast_to((128, 4, S)),
                op=Alu.mult)
            ytp = ps1.tile([64, GPC], bf16, tag="ytp")
            for t4 in range(4):
                nc.tensor.transpose(
                    ytp[:, 128 * t4:128 * (t4 + 1)],
                    ynrm[:, S * t4:S * (t4 + 1)],
                    identb, tile_position=(0, 0))
            ynT = wk.tile([64, GPC], bf16, tag="ynT")
            nc.vector.tensor_copy(ynT[:], ytp[:])
            hT = ps1.tile([H, GPC], f32, tag="hT")
            nc.tensor.matmul(hT[:], Ws[p], ynT[:], start=True, stop=True)
            sg = wk.tile([H, GPC], bf16, tag="sig", name=f"sig_{p}")
            nc.scalar.activation(sg[:], hT[:], Act.Sigmoid, bias=biases[p])
            s["sig"] = sg

        prod = wk.tile([H, GPC], bf16, tag="prod")
        nc.vector.tensor_tensor(prod[:], st["u"]["sig"][:], st["d"]["sig"][:],
                                op=Alu.mult)
        o_ps = ps1.tile([1, GPC], f32, tag="mix")
        nc.tensor.matmul(o_ps[:], mlpW, prod[:], start=True, stop=True)
        o_sb = wk.tile([1, GPC], f32, tag="o_sb")
        nc.vector.tensor_scalar(
            o_sb[:], o_ps[:], mlpb, None, op0=Alu.add)
        nc.sync.dma_start(out_dram.ap(), o_sb[:])

    nc.compile()
    return nc


def _get_module():
    if "nc" not in _CACHE:
        _CACHE["nc"] = _build_module()
    return _CACHE["nc"]


def _perm():
    # device col c = 128*t' + 32*q + 4*m + b  ->  tile 32q+4m+t', block b
    c = np.arange(GPC)
    tp, r = c // 128, c % 128
    q, mm, b = r // 32, (r % 32) // 4, r % 4
    return 4 * (32 * q + 4 * mm + tp) + b


def _prep_branch(x, ei, W, att_src, att_dst, bias):
    """Host-side sharding + graph-format prep for one branch."""
    import ml_dtypes
    bf = ml_dtypes.bfloat16
    x = np.asarray(x, np.float32)
    src = np.asarray(ei[0]).astype(np.int64)
    dst = np.asarray(ei[1]).astype(np.int64)
    W = np.asarray(W, np.float32)
    w_src = (W @ np.asarray(att_src, np.float32)).astype(np.float32)
    w_dst = (W @ np.asarray(att_dst, np.float32)).astype(np.float32)

    valid = (dst % NPG) == (NPG - 1)
    cnt = np.bincount(src[valid], minlength=N).astype(np.float32)

    per_core = []
    for c in range(NC):
        xs = x[c * NPC:(c + 1) * NPC]
        xab = np.zeros((T, 128, SA), np.float32)
        xab[:, :, :S] = xs.reshape(T, 128, S)
        xab[:, :, S] = 1.0
        xab = np.ascontiguousarray(
            xab.transpose(1, 0, 2).reshape(128, T * SA)).astype(bf)
        # xt[64k+s, m] = x[8192k + m, s]
        xtv = xs.reshape(2, NPC // 2, S).transpose(0, 2, 1)
        xtv = np.ascontiguousarray(xtv.reshape(128, NPC // 2)).astype(bf)
        cnt_t = np.ascontiguousarray(
            cnt[c * NPC:(c + 1) * NPC].reshape(T, 128).T)
        xlast = np.ascontiguousarray(
            xs[NPG - 1::NPG].reshape(128, 4 * S)).astype(bf)
        per_core.append({"xab": xab, "xt": xtv, "cnt": cnt_t, "xlast": xlast})

    wv2 = np.zeros((128, 2), np.float32)
    wv2[:S, 0] = w_src
    wv2[S:, 1] = w_src
    wdst_rep = np.broadcast_to(w_dst, (128, 4, S)).reshape(128, 4 * S)
    shared = {
        "wv2": wv2,
        "wdst": wdst_rep.astype(np.float32),
        "W": W,
        "bias": np.asarray(bias, np.float32).reshape(H, 1),
    }
    return per_core, shared


def _build_in_maps(inputs):
    import ml_dtypes
    bf = ml_dtypes.bfloat16
    pcs = {}
    shareds = {}
    pcs["u"], shareds["u"] = _prep_branch(
        inputs["up_x"], inputs["up_edge_index"], inputs["up_W"],
        inputs["up_att_src"], inputs["up_att_dst"], inputs["up_bias"])
    pcs["d"], shareds["d"] = _prep_branch(
        inputs["down_x"], inputs["down_edge_index"], inputs["down_W"],
        inputs["down_att_src"], inputs["down_att_dst"], inputs["down_bias"])

    pp = np.arange(128)
    cf = np.zeros((128, 392), np.float32)
    cf[:, 4] = 1e-16                               # eps
    cf[0, 5] = float(np.asarray(inputs["mlp_b"]).reshape(-1)[0])
    cf[:, 6] = shareds["u"]["bias"][:, 0]
    cf[:, 7] = shareds["d"]["bias"][:, 0]
    cf[:, 8:136] = np.eye(128, dtype=np.float32)

    cbase = np.zeros((128, 1797), np.float32)
    cbase[pp // 32, pp] = 1.0                      # Qm
    cbase[:, 128:384] = shareds["u"]["wdst"]
    cbase[:, 384:640] = shareds["d"]["wdst"]
    cbase[:, 640:642] = shareds["u"]["wv2"]
    cbase[:, 642:644] = shareds["d"]["wv2"]
    for m in range(8):
        cbase[pp, 644 + 32 * m + 4 * m + pp // 32] = 1.0
    cbase[0:64, 900:1028] = shareds["u"]["W"]
    cbase[0:64, 1028:1156] = shareds["d"]["W"]
    cbase[:, 1156] = np.asarray(inputs["mlp_W"], np.float32).reshape(H)
    cbase[:, 1157:1285] = np.eye(128, dtype=np.float32)

    in_maps = []
    for c in range(NC):
        cfc = cf.copy()
        cfc[:, 136:264] = pcs["u"][c]["cnt"]
        cfc[:, 264:392] = pcs["d"][c]["cnt"]
        cbc = cbase.copy()
        cbc[:, 1285:1541] = pcs["u"][c]["xlast"].astype(np.float32)
        cbc[:, 1541:1797] = pcs["d"][c]["xlast"].astype(np.float32)
        m = {"cf": cfc, "cb": cbc.astype(bf)}
        for p in ("u", "d"):
            m[f"{p}_xab"] = pcs[p][c]["xab"]
            m[f"{p}_xt"] = pcs[p][c]["xt"]
        in_maps.append(m)
    return in_maps


def kernel(**inputs):
    from concourse.bass_utils import run_bass_kernel_spmd

    nc = _get_module()
    in_maps = _build_in_maps(inputs)
    res = run_bass_kernel_spmd(nc, in_maps, core_ids=list(range(NC)))
    perm = _perm()
    out = np.empty((NC, GPC), np.float32)
    for c, r in enumerate(res.results):
        out[c, perm] = np.asarray(r["out"], np.float32).reshape(GPC)
    return out.reshape(G, 1)
